# revision 11
# baseline (speedup 1.0000x reference)
"""Trainium2 Bass kernel for ChannelMixingKAN.

Model: LN over (T,C) per batch -> KANLinear(C=128 -> H=256) -> KANLinear(H=256 -> C=128)
with cubic B-spline bases (grid 5, order 3, range [-1,1]) -> residual.

Strategy (v2 — mixed poly/exact batches):
  - Data-parallel over batch: 64 batches -> 8 cores x 8 batches, no collectives.
  - Native (C, T) layout throughout: no transposes.
  - The baseline (v1) computed the 16 exact "tent-cube" spline features
    (m_j^3, n_j^3) with fused custom DVE ops and ran 17 fp16 contraction
    chunks per layer-half.  Measured: DVE ~102us busy and PE ~100us busy per
    iteration -- both engines pegged ("ridge").  To go faster BOTH must drop:

  - POLY batches: each KANLinear's spline term sum_j w_j B_j(z) is replaced by
    a degree-12 polynomial in c = clip(z, -2.2, 2.2)/2.2 (B_j vanish outside
    [-2.2, 2.2], so the clamp gives exact zero-tails up to fit error).  The
    per-(out,in) polynomial coefficients are a fixed linear transform of the
    spline weights (ridge-regularized weighted LSQ fit of the 8 basis
    functions, weighted by the empirical z distribution measured on a
    subsampled CPU forward pass).  Features are c^p computed with STOCK fp16
    DVE multiplies (2x mode) and Act-engine Squares -- ~8x cheaper on DVE than
    the fused custom ops -- but cost 13 fp16 matmul chunks per layer-half.
  - EXACT batches: v1's tent-cube features, emitted by the custom DVE op
    directly in fp8e4m3, and contracted with DoubleRow fp8 matmuls (2 chunks
    per instruction, 2x PE throughput).  DVE-heavy, PE-light.
  - Mixing E exact + (8-E) poly batches per core balances DVE vs PE.
  - fp8 scale: spline weights are multiplied by S=512 so they sit in e4m3's
    normal range; the 1/512 is folded into the activation-engine scale of the
    next stage.  Accuracy (numpy sim): poly-only ~0.6%, exact-fp8 ~0.4%,
    mixed in between -- gate is 2%.
"""

import numpy as np

B, C, T, H = 64, 128, 512, 256
N_CORES = 8
BPC = B // N_CORES
LN_EPS = 1e-5
DEG = 12                      # polynomial degree (chunks = DEG + silu)
SP = 512.0                    # fp8 weight scale for exact batches
CLIP = 2.2

_CACHE = {}
_KAN_CUBE8 = None


def _register_custom_op():
    """Fused tent-cube DVE op: out[p,s,k] = min(|x| - imm2, 0)^3 with
    x = in0[p,s,k] + (s0 + s*s1).  Registered once per process."""
    global _KAN_CUBE8
    if _KAN_CUBE8 is not None:
        return _KAN_CUBE8
    import numpy as _np
    import concourse.dve_ops as DO
    from concourse.dve_spec import (Spec, Src0, C0, C1, C2, Zero, maxx, minn,
                                    sq, PageIdx, lower)
    from concourse.dve_uop import DveOpSpec

    name = "KAN_CUBE8_ANT"
    if name in DO._SUB_OPCODE_FOR_NAME:
        _KAN_CUBE8 = next(op for op in DO.OPS if op.name == name)
        return _KAN_CUBE8
    pg = PageIdx(C0, C1)
    x = Src0 + pg
    v = maxx(x, Zero - x)
    m = minn(v - C2, Zero)

    def ref(in0, in1, s0, s1, imm2):
        S = in0.shape[1] if in0.ndim == 3 else 1
        idx = _np.arange(S).reshape(1, S, 1)
        mm = _np.minimum(_np.abs(in0 + (s0 + idx * s1)) - imm2, 0.0)
        return mm ** 3

    spec = Spec(body=sq(m) * m, reference=ref)
    row = DO._CUSTOM_DVE_ROW_BASE + len(DO.OPS)
    shas = {}
    for ver in ("v3", "v4"):
        dspec = DveOpSpec(name=name, opcode=row, uops=lower(spec, ver=ver),
                          rd1_en=False)
        shas[ver] = dspec.sha(ver)
    op = DO.DveOp(name=name, spec=spec, subdim=True, uops_sha=shas)
    DO.OPS.append(op)
    DO.CUSTOM_DVE_SPECS[name] = spec
    DO._SUB_OPCODE_FOR_NAME[name] = row
    _KAN_CUBE8 = op
    return op


# ---------------------------------------------------------------- host math
def _make_grid(nf):
    h = 2.0 / 5
    g = np.arange(-3, 5 + 3 + 1, dtype=np.float64) * h - 1.0
    return np.broadcast_to(g, (nf, g.shape[0]))


def _b_splines(x, grid):
    xg = x[:, :, None]
    bases = ((xg >= grid[:, :-1]) & (xg < grid[:, 1:])).astype(x.dtype)
    for k in range(1, 4):
        left = (xg - grid[:, :-(k + 1)]) / (grid[:, k:-1] - grid[:, :-(k + 1)])
        right = (grid[:, k + 1:] - xg) / (grid[:, k + 1:] - grid[:, 1:-k])
        bases = left * bases[:, :, :-1] + right * bases[:, :, 1:]
    return bases


def _silu(x):
    return x / (1 + np.exp(-x))


def _fit_beta(deg, samples, lam=1e-6):
    """Weighted ridge fit of the 8 basis functions by polynomials in
    (z/CLIP)^p, p=0..deg.  Weight = empirical density of `samples`."""
    zs = np.linspace(-CLIP, CLIP, 4001)
    Bz = _b_splines(zs[:, None], _make_grid(1))[:, 0, :]          # (N, 8)
    s = np.clip(samples, -CLIP, CLIP)
    hist, edges = np.histogram(s, bins=200, range=(-CLIP, CLIP), density=True)
    wz = np.interp(zs, 0.5 * (edges[1:] + edges[:-1]), hist) + 1e-4
    X = np.stack([(zs / CLIP) ** p for p in range(deg + 1)], axis=1)
    W = np.sqrt(wz)[:, None]
    A = X * W
    beta = np.linalg.solve(A.T @ A + lam * np.eye(deg + 1), A.T @ (Bz * W))
    return beta                                                    # (deg+1, 8)


def _prep_weights(U, bw1, sw1, ss1, bw2, sw2, ss2):
    """All weight tensors in on-chip (lhsT) layout.  Includes a subsampled CPU
    forward pass to get the empirical z1/z2 distributions for the poly fit."""
    f64 = np.float64
    sws1 = (sw1 * ss1[:, :, None]).astype(f64)                     # (H, C, 8)
    sws2 = (sw2 * ss2[:, :, None]).astype(f64)                     # (C, H, 8)

    # ---- empirical z1 / z2 (T subsampled x8) ----
    z = np.transpose(U, (0, 2, 1)).astype(f64)                     # (B, T, C)
    mu = z.mean(axis=(-2, -1), keepdims=True)
    var = z.var(axis=(-2, -1), keepdims=True)
    z1 = ((z - mu) / np.sqrt(var + LN_EPS))[:, ::8, :].reshape(-1, C)
    g1 = _make_grid(C)
    z2 = (_silu(z1) @ bw1.T.astype(f64) +
          np.einsum('nic,oic->no', _b_splines(z1, g1), sws1))      # (N, H)

    beta1 = _fit_beta(DEG, z1.reshape(-1))                         # (13, 8)
    beta2 = _fit_beta(DEG, z2.reshape(-1))

    # ---- poly weights: a[o, i, p] = sum_j sws[o,i,j] beta[p,j] ----
    a1 = np.einsum('oij,pj->oip', sws1, beta1)                     # (H, C, 13)
    a2 = np.einsum('oij,pj->oip', sws2, beta2)                     # (C, H, 13)
    b1 = a1[:, :, 0].sum(axis=1)                                   # (H,)
    b2 = a2[:, :, 0].sum(axis=1)                                   # (C,)
    # lhsT layouts
    a1_l = np.ascontiguousarray(np.transpose(a1[:, :, 1:], (1, 2, 0))
                                ).astype(np.float16)               # (C,12,H)
    a2_l = np.ascontiguousarray(
        a2[:, :, 1:].reshape(C, 2, 128, DEG).transpose(2, 1, 3, 0)
    ).astype(np.float16)                                           # (128,2,12,C)
    bw1p = np.ascontiguousarray(bw1.T).astype(np.float16)          # (C, H)
    bw2p = np.ascontiguousarray(
        bw2.reshape(C, 2, 128).transpose(2, 1, 0)).astype(np.float16)  # (128,2,C)
    b1_l = np.ascontiguousarray(b1.reshape(2, 128).T).astype(np.float32)  # (128,2)
    b1c_l = (b1_l / CLIP).astype(np.float32)                       # (128,2)
    b2_l = b2.reshape(C, 1).astype(np.float32)                     # (128,1)

    # ---- exact weights (x SP), fp8 tent-cube chunks ----
    import ml_dtypes
    F8 = ml_dtypes.float8_e4m3fn
    w1s = (bw1.T * SP).astype(np.float16)                          # (C, H)
    w2s = np.ascontiguousarray(
        (bw2 * SP).reshape(C, 2, 128).transpose(2, 1, 0)).astype(np.float16)
    # chunk order per layer: [m0..m7, n0..n7]; weight for m-chunk j:
    # -sws[:, :, j]/6 * SP ; n-chunk j: +(2/3) sws[:, :, j] * SP
    wch1 = np.empty((16, C, H), f64)
    wch2 = np.empty((16, H, C), f64)
    for j in range(8):
        wch1[j] = (-1 / 6) * SP * sws1[:, :, j].T
        wch1[8 + j] = (2 / 3) * SP * sws1[:, :, j].T
        wch2[j] = (-1 / 6) * SP * sws2[:, :, j].T
        wch2[8 + j] = (2 / 3) * SP * sws2[:, :, j].T
    # w1dr[c, pair, two, half, m] = wch1[2*pair+two][c, half*128 + m]
    w1dr = np.ascontiguousarray(
        wch1.reshape(8, 2, C, 2, 128).transpose(2, 0, 1, 3, 4)).astype(F8)
    # w2dr[hpart, Hhalf, pair, two, cout] = wch2[2*pair+two][Hhalf*128+hpart, cout]
    w2dr = np.ascontiguousarray(
        wch2.reshape(8, 2, 2, 128, C).transpose(3, 2, 0, 1, 4)).astype(F8)

    return dict(a1=a1_l, a2=a2_l, bw1p=bw1p, bw2p=bw2p, b1=b1_l, b1c=b1c_l,
                b2=b2_l, w1s=w1s, w2s=w2s, w1dr=w1dr, w2dr=w2dr)


# ---------------------------------------------------------------- bass build
def _build(ln_affine: bool, sim_safe: bool = False, trace_sim: bool = False,
           n_loop: int = 1, n_exact: int = 3):
    import concourse.bacc as bacc
    import concourse.tile as tile
    from concourse import mybir
    from contextlib import ExitStack, nullcontext
    import concourse.bass as bass_mod

    F16 = mybir.dt.float16
    F32 = mybir.dt.float32
    F8 = mybir.dt.float8e4
    Op = mybir.AluOpType
    Act = mybir.ActivationFunctionType
    DR = mybir.MatmulPerfMode.DoubleRow
    ACT_SILU = Act.Sigmoid if sim_safe else Act.Silu

    nc = bacc.Bacc(None, target_bir_lowering=False)

    u_ext = nc.declare_dram_parameter("u", [BPC, C, T], F32, isOutput=False)
    a1_ext = nc.declare_dram_parameter("a1", [C, DEG, H], F16, isOutput=False)
    a2_ext = nc.declare_dram_parameter("a2", [128, 2, DEG, C], F16, isOutput=False)
    bw1p_ext = nc.declare_dram_parameter("bw1p", [C, H], F16, isOutput=False)
    bw2p_ext = nc.declare_dram_parameter("bw2p", [128, 2, C], F16, isOutput=False)
    b1_ext = nc.declare_dram_parameter("b1", [128, 2], F32, isOutput=False)
    b1c_ext = nc.declare_dram_parameter("b1c", [128, 2], F32, isOutput=False)
    b2_ext = nc.declare_dram_parameter("b2", [128, 1], F32, isOutput=False)
    if n_exact > 0:
        w1s_ext = nc.declare_dram_parameter("w1s", [C, H], F16, isOutput=False)
        w2s_ext = nc.declare_dram_parameter("w2s", [128, 2, C], F16, isOutput=False)
        w1dr_ext = nc.declare_dram_parameter("w1dr", [C, 8, 2, 2, 128], F8, isOutput=False)
        w2dr_ext = nc.declare_dram_parameter("w2dr", [128, 2, 8, 2, C], F8, isOutput=False)
    if ln_affine:
        lnw_ext = nc.declare_dram_parameter("lnw", [C, T], F32, isOutput=False)
        lnb_ext = nc.declare_dram_parameter("lnb", [C, T], F32, isOutput=False)
    out_ext = nc.declare_dram_parameter("out", [BPC, C, T], F32, isOutput=True)

    kc = _register_custom_op() if n_exact > 0 else None

    with tile.TileContext(nc, trace_sim=trace_sim) as tc, ExitStack() as ctx:
        singles = ctx.enter_context(tc.tile_pool(name="singles", bufs=1))
        u_pool = ctx.enter_context(tc.tile_pool(name="u", bufs=BPC))
        st_pool = ctx.enter_context(tc.tile_pool(name="st", bufs=2))
        z_pool = ctx.enter_context(tc.tile_pool(name="z", bufs=4))
        f1_pool = ctx.enter_context(tc.tile_pool(name="f1", bufs=2))
        f2_pool = ctx.enter_context(tc.tile_pool(name="f2", bufs=4))
        sil_pool = ctx.enter_context(tc.tile_pool(name="sil", bufs=4))
        e1_pool = ctx.enter_context(tc.tile_pool(name="e1", bufs=2))
        e2_pool = ctx.enter_context(tc.tile_pool(name="e2", bufs=3))
        o_pool = ctx.enter_context(tc.tile_pool(name="o", bufs=3))
        psum = ctx.enter_context(tc.tile_pool(name="psum", bufs=5, space="PSUM"))
        psum2 = ctx.enter_context(tc.tile_pool(name="psum2", bufs=2, space="PSUM"))
        psum_s = ctx.enter_context(tc.tile_pool(name="psum_s", bufs=1, space="PSUM"))

        # ---- weights / constants (outside the timing loop) ----
        a1_sb = singles.tile([C, DEG, H], F16)
        nc.sync.dma_start(out=a1_sb[:], in_=a1_ext[:])
        a2_sb = singles.tile([128, 2, DEG, C], F16)
        nc.sync.dma_start(out=a2_sb[:], in_=a2_ext[:])
        bw1p_sb = singles.tile([C, H], F16)
        nc.sync.dma_start(out=bw1p_sb[:], in_=bw1p_ext[:])
        bw2p_sb = singles.tile([128, 2, C], F16)
        nc.sync.dma_start(out=bw2p_sb[:], in_=bw2p_ext[:])
        b1_sb = singles.tile([128, 2], F32)
        nc.sync.dma_start(out=b1_sb[:], in_=b1_ext[:])
        b1c_sb = singles.tile([128, 2], F32)
        nc.sync.dma_start(out=b1c_sb[:], in_=b1c_ext[:])
        b2_sb = singles.tile([128, 1], F32)
        nc.sync.dma_start(out=b2_sb[:], in_=b2_ext[:])
        if n_exact > 0:
            w1s_sb = singles.tile([C, H], F16)
            nc.sync.dma_start(out=w1s_sb[:], in_=w1s_ext[:])
            w2s_sb = singles.tile([128, 2, C], F16)
            nc.sync.dma_start(out=w2s_sb[:], in_=w2s_ext[:])
            w1dr_sb = singles.tile([C, 8, 2, 2, 128], F8)
            nc.sync.dma_start(out=w1dr_sb[:], in_=w1dr_ext[:])
            w2dr_sb = singles.tile([128, 2, 8, 2, C], F8)
            nc.sync.dma_start(out=w2dr_sb[:], in_=w2dr_ext[:])
        if ln_affine:
            lnw_sb = singles.tile([C, T], F32)
            nc.sync.dma_start(out=lnw_sb[:], in_=lnw_ext[:])
            lnb_sb = singles.tile([C, T], F32)
            nc.sync.dma_start(out=lnb_sb[:], in_=lnb_ext[:])
        ones_sb = singles.tile([128, 128], F32)
        nc.vector.memset(ones_sb[:], 1.0 / 128.0)
        eps_sb = singles.tile([128, 1], F32)
        nc.vector.memset(eps_sb[:], LN_EPS)
        coef = singles.tile([128, BPC, 6], F32)   # [inv, a_s, b_s, nb, inv/CLIP, nb/CLIP]

        def s_bcast(s_tile, S):
            ap = s_tile[:]
            return bass_mod.AP(tensor=ap.tensor, offset=ap.offset,
                               ap=[ap.ap[0], [0, S], ap.ap[1]])

        loop_cm = tc.For_i(0, n_loop, 1) if n_loop > 1 else nullcontext()
        with loop_cm:
            # ================= LN stats (grouped, as v1) =================
            u_tiles = []
            GSZ = 8
            for g0 in range(0, BPC, GSZ):
                gn = min(GSZ, BPC - g0)
                mv_all = st_pool.tile([128, gn, 2], F32, tag="mv_all")
                for i in range(gn):
                    b = g0 + i
                    u_t = u_pool.tile([C, T], F32, tag="u")
                    nc.sync.dma_start(out=u_t[:], in_=u_ext[b])
                    u_tiles.append(u_t)
                    st = st_pool.tile([128, 6], F32, tag="bnst")
                    nc.vector.bn_stats(out=st[:], in_=u_t[:])
                    nc.vector.bn_aggr(out=mv_all[:, i, :], in_=st[:])
                cf = coef[:, g0:g0 + gn, :]
                m2a = st_pool.tile([128, gn, 1], F32, tag="m2a")
                nc.vector.tensor_tensor(out=m2a[:], in0=mv_all[:, :, 0:1],
                                        in1=mv_all[:, :, 0:1], op=Op.mult)
                nc.vector.tensor_tensor(out=mv_all[:, :, 1:2], in0=mv_all[:, :, 1:2],
                                        in1=m2a[:], op=Op.add)
                ps = psum_s.tile([128, gn, 2], F32, tag="ps_st")
                nc.tensor.matmul(ps[:], lhsT=ones_sb[:], rhs=mv_all[:], start=True, stop=True)
                stot = st_pool.tile([128, gn, 2], F32, tag="stot")
                nc.vector.tensor_copy(out=stot[:], in_=ps[:])
                var = st_pool.tile([128, gn, 1], F32, tag="var")
                nc.vector.tensor_tensor(out=var[:], in0=stot[:, :, 0:1],
                                        in1=stot[:, :, 0:1], op=Op.mult)
                nc.vector.tensor_tensor(out=var[:], in0=stot[:, :, 1:2], in1=var[:],
                                        op=Op.subtract)
                sd = st_pool.tile([128, gn, 1], F32, tag="sd")
                nc.scalar.activation(out=sd[:], in_=var[:], func=Act.Sqrt, bias=eps_sb[:])
                nc.vector.reciprocal(out=cf[:, :, 0:1], in_=sd[:])
                mi = st_pool.tile([128, gn, 1], F32, tag="mi")
                nc.vector.tensor_tensor(out=mi[:], in0=stot[:, :, 0:1],
                                        in1=cf[:, :, 0:1], op=Op.mult)
                nc.vector.tensor_scalar(out=cf[:, :, 1:2], in0=cf[:, :, 0:1],
                                        scalar1=2.5, scalar2=None, op0=Op.mult)
                nc.vector.tensor_scalar(out=cf[:, :, 2:3], in0=mi[:], scalar1=-2.5,
                                        scalar2=5.5, op0=Op.mult, op1=Op.add)
                nc.vector.tensor_scalar(out=cf[:, :, 3:4], in0=mi[:], scalar1=-1.0,
                                        scalar2=None, op0=Op.mult)
                nc.vector.tensor_scalar(out=cf[:, :, 4:5], in0=cf[:, :, 0:1],
                                        scalar1=1.0 / CLIP, scalar2=None, op0=Op.mult)
                nc.vector.tensor_scalar(out=cf[:, :, 5:6], in0=mi[:],
                                        scalar1=-1.0 / CLIP, scalar2=None, op0=Op.mult)

            # ================= helpers =================
            def poly_features(feats):
                """feats[:, 1, :] = c already set; fill feats[:, 2..DEG, :] = c^p
                (fp16).  Even powers on Act (Square), odd on DVE (mult)."""
                c = feats[:, 1, :]
                # evens via Act Square: c2=S(c), c4=S(c2), c6=S(c3), c8=S(c4),
                # c10=S(c5), c12=S(c6); odds via DVE: c3=c2*c, c5=c4*c, ...
                nc.scalar.activation(out=feats[:, 2, :], in_=c, func=Act.Square)
                nc.vector.tensor_tensor(out=feats[:, 3, :], in0=feats[:, 2, :],
                                        in1=c, op=Op.mult)
                nc.scalar.activation(out=feats[:, 4, :], in_=feats[:, 2, :], func=Act.Square)
                nc.vector.tensor_tensor(out=feats[:, 5, :], in0=feats[:, 4, :],
                                        in1=c, op=Op.mult)
                nc.scalar.activation(out=feats[:, 6, :], in_=feats[:, 3, :], func=Act.Square)
                nc.vector.tensor_tensor(out=feats[:, 7, :], in0=feats[:, 6, :],
                                        in1=c, op=Op.mult)
                nc.scalar.activation(out=feats[:, 8, :], in_=feats[:, 4, :], func=Act.Square)
                nc.vector.tensor_tensor(out=feats[:, 9, :], in0=feats[:, 8, :],
                                        in1=c, op=Op.mult)
                nc.scalar.activation(out=feats[:, 10, :], in_=feats[:, 5, :], func=Act.Square)
                nc.vector.tensor_tensor(out=feats[:, 11, :], in0=feats[:, 10, :],
                                        in1=c, op=Op.mult)
                nc.scalar.activation(out=feats[:, 12, :], in_=feats[:, 6, :], func=Act.Square)

            # ================= per-batch =================
            for b in range(BPC):
                u_t = u_tiles[b]
                inv = coef[:, b, 0:1]
                a_s = coef[:, b, 1:2]
                b_s = coef[:, b, 2:3]
                nb = coef[:, b, 3:4]
                exact = b < n_exact

                if ln_affine:
                    zln = z_pool.tile([128, T], F32, tag="zln")
                    nc.vector.tensor_scalar(out=zln[:], in0=u_t[:], scalar1=inv,
                                            scalar2=nb, op0=Op.mult, op1=Op.add)
                    nc.vector.tensor_tensor(out=zln[:], in0=zln[:], in1=lnw_sb[:], op=Op.mult)
                    nc.vector.tensor_tensor(out=zln[:], in0=zln[:], in1=lnb_sb[:], op=Op.add)

                if exact:
                    # ---------------- exact batch (fp8 tent-cubes + DR) -------
                    s1 = z_pool.tile([128, T], F32, tag="s1")
                    sil1 = sil_pool.tile([128, T], F16, tag="sil1")
                    if ln_affine:
                        nc.vector.tensor_scalar(out=s1[:], in0=zln[:], scalar1=2.5,
                                                scalar2=5.5, op0=Op.mult, op1=Op.add)
                        nc.scalar.activation(out=sil1[:], in_=zln[:], func=ACT_SILU)
                    else:
                        nc.vector.tensor_scalar(out=s1[:], in0=u_t[:], scalar1=a_s,
                                                scalar2=b_s, op0=Op.mult, op1=Op.add)
                        nc.scalar.activation(out=sil1[:], in_=u_t[:], func=ACT_SILU,
                                             bias=nb, scale=inv)
                    sp1 = e1_pool.tile([128, 16, T], F8, tag="sp1")
                    sb1 = s_bcast(s1, 8)
                    nc.vector._custom_dve(kc, out=sp1[:, 0:8, :], in0=sb1,
                                          s0=-2.0, s1=-1.0, imm2=2.0)
                    nc.vector._custom_dve(kc, out=sp1[:, 8:16, :], in0=sb1,
                                          s0=-2.0, s1=-1.0, imm2=1.0)
                    sil2 = []
                    sp2 = []
                    for h in range(2):
                        ps1 = psum.tile([128, T], F32, tag="ps1")
                        nc.tensor.matmul(ps1[:], lhsT=w1s_sb[:, h * 128:(h + 1) * 128],
                                         rhs=sil1[:], start=True, stop=False)
                        for pr in range(8):
                            nc.tensor.matmul(ps1[:], lhsT=w1dr_sb[:, pr, :, h, :],
                                             rhs=sp1[:, 2 * pr:2 * pr + 2, :],
                                             perf_mode=DR,
                                             start=False, stop=(pr == 7))
                        s2h = z_pool.tile([128, T], F32, tag="s2")
                        nc.scalar.activation(out=s2h[:], in_=ps1[:], func=Act.Copy,
                                             bias=5.5, scale=2.5 / SP)
                        sl = sil_pool.tile([128, T], F16, tag="sil2")
                        nc.scalar.activation(out=sl[:], in_=ps1[:], func=ACT_SILU,
                                             scale=1.0 / SP)
                        sil2.append(sl)
                        spx = e2_pool.tile([128, 16, T], F8, tag="sp2")
                        sb2 = s_bcast(s2h, 8)
                        nc.vector._custom_dve(kc, out=spx[:, 0:8, :], in0=sb2,
                                              s0=-2.0, s1=-1.0, imm2=2.0)
                        nc.vector._custom_dve(kc, out=spx[:, 8:16, :], in0=sb2,
                                              s0=-2.0, s1=-1.0, imm2=1.0)
                        sp2.append(spx)
                    ps2 = psum2.tile([128, T], F32, tag="ps2")
                    for h in range(2):
                        nc.tensor.matmul(ps2[:], lhsT=w2s_sb[:, h, :], rhs=sil2[h][:],
                                         start=(h == 0), stop=False)
                    for h in range(2):
                        for pr in range(8):
                            nc.tensor.matmul(ps2[:], lhsT=w2dr_sb[:, h, pr, :, :],
                                             rhs=sp2[h][:, 2 * pr:2 * pr + 2, :],
                                             perf_mode=DR, start=False,
                                             stop=(h == 1 and pr == 7))
                    o1 = o_pool.tile([128, T], F32, tag="o1")
                    nc.scalar.activation(out=o1[:], in_=ps2[:], func=Act.Copy,
                                         scale=1.0 / SP)
                    o_t = o_pool.tile([128, T], F32, tag="o")
                    nc.vector.tensor_tensor(out=o_t[:], in0=o1[:], in1=u_t[:], op=Op.add)
                    nc.sync.dma_start(out=out_ext[b], in_=o_t[:])
                else:
                    # ---------------- poly batch ------------------------------
                    feats1 = f1_pool.tile([128, 1 + DEG, T], F16, tag="feats1")
                    tmp1 = z_pool.tile([128, T], F16, tag="tmp1")
                    if ln_affine:
                        nc.vector.tensor_scalar(out=tmp1[:], in0=zln[:],
                                                scalar1=1.0 / CLIP, scalar2=None,
                                                op0=Op.mult)
                        nc.scalar.activation(out=feats1[:, 0, :], in_=zln[:], func=ACT_SILU)
                    else:
                        nc.vector.tensor_scalar(out=tmp1[:], in0=u_t[:],
                                                scalar1=coef[:, b, 4:5],
                                                scalar2=coef[:, b, 5:6],
                                                op0=Op.mult, op1=Op.add)
                        nc.scalar.activation(out=feats1[:, 0, :], in_=u_t[:], func=ACT_SILU,
                                             bias=nb, scale=inv)
                    nc.vector.tensor_scalar(out=feats1[:, 1, :], in0=tmp1[:],
                                            scalar1=1.0, scalar2=-1.0,
                                            op0=Op.min, op1=Op.max)
                    poly_features(feats1)
                    feats2 = []
                    for h in range(2):
                        ps1 = psum.tile([128, T], F32, tag="ps1")
                        nc.tensor.matmul(ps1[:], lhsT=bw1p_sb[:, h * 128:(h + 1) * 128],
                                         rhs=feats1[:, 0, :], start=True, stop=False)
                        for p in range(DEG):
                            nc.tensor.matmul(ps1[:], lhsT=a1_sb[:, p, h * 128:(h + 1) * 128],
                                             rhs=feats1[:, 1 + p, :],
                                             start=False, stop=(p == DEG - 1))
                        f2 = f2_pool.tile([128, 1 + DEG, T], F16, tag="feats2")
                        nc.scalar.activation(out=f2[:, 0, :], in_=ps1[:], func=ACT_SILU,
                                             bias=b1_sb[:, h:h + 1])
                        tmp2 = z_pool.tile([128, T], F16, tag="tmp2")
                        nc.vector.tensor_scalar(out=tmp2[:], in0=ps1[:],
                                                scalar1=1.0 / CLIP,
                                                scalar2=b1c_sb[:, h:h + 1],
                                                op0=Op.mult, op1=Op.add)
                        nc.vector.tensor_scalar(out=f2[:, 1, :], in0=tmp2[:],
                                                scalar1=1.0, scalar2=-1.0,
                                                op0=Op.min, op1=Op.max)
                        poly_features(f2)
                        feats2.append(f2)
                    ps2 = psum2.tile([128, T], F32, tag="ps2")
                    for h in range(2):
                        nc.tensor.matmul(ps2[:], lhsT=bw2p_sb[:, h, :],
                                         rhs=feats2[h][:, 0, :],
                                         start=(h == 0), stop=False)
                    for h in range(2):
                        for p in range(DEG):
                            nc.tensor.matmul(ps2[:], lhsT=a2_sb[:, h, p, :],
                                             rhs=feats2[h][:, 1 + p, :],
                                             start=False, stop=(h == 1 and p == DEG - 1))
                    o1 = o_pool.tile([128, T], F32, tag="o1")
                    nc.vector.tensor_scalar(out=o1[:], in0=ps2[:],
                                            scalar1=b2_sb[:, 0:1], scalar2=None,
                                            op0=Op.add)
                    o_t = o_pool.tile([128, T], F32, tag="o")
                    nc.vector.tensor_tensor(out=o_t[:], in0=o1[:], in1=u_t[:], op=Op.add)
                    nc.sync.dma_start(out=out_ext[b], in_=o_t[:])

    nc.compile()
    return nc


def _get_nc(ln_affine: bool, sim_safe: bool = False, trace_sim: bool = False,
            n_loop: int = 1, n_exact: int = 3):
    key = ("nc", ln_affine, sim_safe, trace_sim, n_loop, n_exact)
    if key not in _CACHE:
        _CACHE[key] = _build(ln_affine, sim_safe, trace_sim, n_loop, n_exact)
    return _CACHE[key]


def make_in_maps(U, ln_w, ln_b, bw1, sw1, ss1, bw2, sw2, ss2):
    U = np.ascontiguousarray(np.asarray(U, dtype=np.float32))
    ln_affine = not (np.all(ln_w == 1.0) and np.all(ln_b == 0.0))
    w = _prep_weights(U, np.asarray(bw1, np.float64), np.asarray(sw1, np.float64),
                      np.asarray(ss1, np.float64), np.asarray(bw2, np.float64),
                      np.asarray(sw2, np.float64), np.asarray(ss2, np.float64))
    shards = U.reshape(N_CORES, BPC, C, T)
    in_maps = []
    for c in range(N_CORES):
        m = {"u": shards[c], **w}
        if ln_affine:
            m["lnw"] = np.ascontiguousarray(np.asarray(ln_w, np.float32).T)
            m["lnb"] = np.ascontiguousarray(np.asarray(ln_b, np.float32).T)
        in_maps.append(m)
    return in_maps, ln_affine


def run_in_maps(in_maps, ln_affine, n_loop: int = 1, n_exact: int = 3):
    from concourse.bass_utils import run_bass_kernel_spmd
    nc = _get_nc(ln_affine, n_loop=n_loop, n_exact=n_exact)
    res = run_bass_kernel_spmd(nc, in_maps, core_ids=list(range(N_CORES)))
    return res


def kernel(U, ln_w, ln_b, bw1, sw1, ss1, bw2, sw2, ss2):
    in_maps, ln_affine = make_in_maps(U, ln_w, ln_b, bw1, sw1, ss1, bw2, sw2, ss2)
    res = run_in_maps(in_maps, ln_affine)
    out = np.concatenate([res.results[c]["out"] for c in range(N_CORES)], axis=0)
    return out.reshape(B, C, T).astype(np.float32)
